# revision 1
# baseline (speedup 1.0000x reference)
"""Trainium2 Bass kernel for DensityCalculator.

density[g] = sum_a sum_k aw[a,k]*exp(bw[a,k]*|g-x_a|^2) over a 64^3 grid,
then 3D FFT -> hamming filter -> inverse FFT -> normalize.

Distribution: grid x-axis sharded over 8 cores (8 x-planes each) for the
density phase; FFT is a distributed pencil decomposition (z,y transforms
local to the x-slab, AllToAll to kz-slabs, x transform + filter + inverse x,
AllToAll back, inverse y,z local).

Device algebra:
 - d2 via TensorE: [gx,gy,gz,1,|g|^2] . [-2X; |X|^2; 1]  (K=5 matmul, f32r)
 - aw*exp(bw*d2) in ONE scalar-engine op per k: Exp with per-partition
   scale=bw[:,k], bias=ln(aw[:,k])  (requires aw >= 0, true for this model)
 - sum over atoms via ones-vector matmul accumulated over k in PSUM
 - FFT as matmuls with 64x64 DFT matrices (fftshift/ifftshift folded into
   column/row permutations host-side), transposes on TensorE.
"""

import os
import sys
import numpy as np

for _p in ("/opt/trn_rl_repo", "/root/.axon_site", "/root/.axon_site/_ro/trn_rl_repo",
           "/root/.axon_site/_ro/pypackages"):
    if _p not in sys.path and os.path.isdir(_p):
        sys.path.append(_p)

import concourse.bass as bass
import concourse.tile as tile
from concourse import bacc, mybir
from concourse.bass_utils import run_bass_kernel_spmd

FP = mybir.dt.float32
FR = mybir.dt.float32r
BF = mybir.dt.bfloat16
Exp = mybir.ActivationFunctionType.Exp

N_CORES = 8
N = 64              # grid size per axis
A = 128             # atoms
K = 6               # gaussian terms
XL = 8              # x-planes per core
GPC = N * XL * N    # grid points per core (32768)
NG = 8              # phase-1 groups per core
GSZ = GPC // NG     # 4096 points per group

LAST_EXEC_NS = None
LAST_RESULTS = None
_COMPILED = None
DEBUG_TAPS = bool(os.environ.get("KERNEL_DEBUG_TAPS"))


def _build():
    nc = bacc.Bacc("TRN2", target_bir_lowering=False, debug=False,
                   num_devices=N_CORES)

    rhs5 = nc.dram_tensor("rhs5", [5, GPC], FP, kind="ExternalInput").ap()
    xstat = nc.dram_tensor("xstat", [5, A], FP, kind="ExternalInput").ap()
    bwln = nc.dram_tensor("bwln", [A, 2 * K], FP, kind="ExternalInput").ap()
    mats = nc.dram_tensor("mats", [N, 128 + 7 * N], FR, kind="ExternalInput").ap()
    ham = nc.dram_tensor("ham", [N, 512], FR, kind="ExternalInput").ap()
    mats2 = nc.dram_tensor("mats2", [8, 8 * 128], FR, kind="ExternalInput").ap()
    out = nc.dram_tensor("out", [N, 512], FP, kind="ExternalOutput").ap()
    taps = {}
    if DEBUG_TAPS:
        for nm in ("dbg_rho", "dbg_f1re", "dbg_f1im", "dbg_t1re", "dbg_t1im",
                   "dbg_f2re", "dbg_f2im", "dbg_g2re", "dbg_g2im",
                   "dbg_t2re", "dbg_t2im", "dbg_f3re", "dbg_f3im",
                   "dbg_g5re", "dbg_g5im"):
            taps[nm] = nc.dram_tensor(nm, [N, 512], FR, kind="ExternalOutput").ap()

    def tap(nm, tile_):
        if DEBUG_TAPS:
            nc.sync.dma_start(taps[nm], tile_[:])

    with tile.TileContext(nc) as tc:
        with tc.tile_pool(name="const", bufs=1) as constp, \
             tc.tile_pool(name="dram", bufs=1, space="DRAM") as dram:
            xstat_sb = constp.tile([5, A], FP)
            nc.sync.dma_start(xstat_sb[:], xstat[:])
            bwln_sb = constp.tile([A, 2 * K], FP)
            nc.sync.dma_start(bwln_sb[:], bwln[:])
            mats_sb = constp.tile([N, 128 + 7 * N], FR)
            nc.sync.dma_start(mats_sb[:], mats[:])
            ham_sb = constp.tile([N, 512], FR)
            nc.sync.dma_start(ham_sb[:], ham[:])
            mats2_sb = constp.tile([8, 8 * 128], FR)
            nc.sync.dma_start(mats2_sb[:], mats2[:])
            # e8[:, j*8:(j+1)*8] is a (128,8) stationary whose col j is all-ones:
            # chunk j's atom-sum matmul lands on psum partition j.
            e8 = constp.tile([A, 64], BF)
            nc.vector.memset(e8[:], 0.0)
            for j in range(8):
                nc.vector.memset(e8[:, j * 8 + j:j * 8 + j + 1], 1.0)

            # tiny warmup AllToAll so ncfw channel setup overlaps phase 1
            wu_in = dram.tile([N_CORES, 8], FP, tag="wu_in")
            wu_out = dram.tile([N_CORES, 8], FP, tag="wu_out")
            wu_sb = constp.tile([1, N_CORES * 8], FP)
            nc.vector.memset(wu_sb[:], 0.0)
            nc.sync.dma_start(wu_in[:].rearrange("a b -> (a b)"), wu_sb[0, :])
            nc.gpsimd.collective_compute(
                "AllToAll", mybir.AluOpType.bypass,
                replica_groups=[list(range(N_CORES))],
                ins=[wu_in.opt()], outs=[wu_out.opt()])

            # stationary views into mats
            Az2T = mats_sb[:, 0:128]
            c0 = 128
            ArT = mats_sb[:, c0:c0 + N]
            AiT = mats_sb[:, c0 + N:c0 + 2 * N]
            AiTn = mats_sb[:, c0 + 2 * N:c0 + 3 * N]
            BrT = mats_sb[:, c0 + 3 * N:c0 + 4 * N]
            BiT = mats_sb[:, c0 + 4 * N:c0 + 5 * N]
            BiTn = mats_sb[:, c0 + 5 * N:c0 + 6 * N]
            ident = mats_sb[:, c0 + 6 * N:c0 + 7 * N]

            # ---------------- Phase 1: density ----------------
            acc_pool = tc.tile_pool(name="p1acc", bufs=1, space="PSUM")
            accps = acc_pool.__enter__().tile([128, 512], FP, tag="acc")
            with tc.tile_pool(name="p1sb", bufs=2) as p1sb, \
                 tc.tile_pool(name="p1e", bufs=1) as p1e, \
                 tc.tile_pool(name="p1ps", bufs=1, space="PSUM") as p1ps, \
                 tc.tile_pool(name="p1rho", bufs=3, space="PSUM") as p1rho:
                def emit_d2(g):
                    """PE: 8 K=5 fp32 matmuls -> PSUM; DVE: copy to SBUF."""
                    rh = p1sb.tile([5, GSZ], FP, tag="rh")
                    nc.sync.dma_start(rh[:], rhs5[:, g * GSZ:(g + 1) * GSZ])
                    d2sb = p1sb.tile([A, GSZ], FP, tag="d2")
                    for h in range(2):
                        d2ps = p1ps.tile([A, 2048], FP, tag="d2ps")
                        for j in range(4):
                            nc.tensor.matmul(
                                d2ps[:, j * 512:(j + 1) * 512],
                                lhsT=xstat_sb[:],
                                rhs=rh[:, h * 2048 + j * 512:h * 2048 + (j + 1) * 512],
                                start=True, stop=True)
                        nc.vector.tensor_copy(d2sb[:, h * 2048:(h + 1) * 2048], d2ps[:])
                    return d2sb

                # software pipeline: emit group g+1's d2 matmuls BEFORE group
                # g's atom-sum matmuls so the PE queue never stalls on scalar.
                d2_next = emit_d2(0)
                for g in range(NG):
                    d2sb = d2_next
                    Es = []
                    for k in range(K):
                        Ek = p1e.tile([A, GSZ], BF, tag=f"E{k}")
                        if g == 0:
                            # split so scalar starts on the first PSUM half sooner
                            for h in range(2):
                                nc.scalar.activation(Ek[:, h * 2048:(h + 1) * 2048],
                                                     d2sb[:, h * 2048:(h + 1) * 2048], Exp,
                                                     bias=bwln_sb[:, K + k:K + k + 1],
                                                     scale=bwln_sb[:, k:k + 1])
                        else:
                            nc.scalar.activation(Ek[:], d2sb[:], Exp,
                                                 bias=bwln_sb[:, K + k:K + k + 1],
                                                 scale=bwln_sb[:, k:k + 1])
                        Es.append(Ek)
                    if g + 1 < NG:
                        d2_next = emit_d2(g + 1)
                    # pair-sum the six E_k on DVE (bf16) to cut atom-sum matmuls 2x
                    Ps = []
                    for i in range(3):
                        Pi = p1e.tile([A, GSZ], BF, tag=f"P{i}")
                        nc.vector.tensor_tensor(Pi[:], Es[2 * i][:], Es[2 * i + 1][:],
                                                op=mybir.AluOpType.add)
                        Ps.append(Pi)
                    rps = p1rho.tile([8, 512], FP, tag="rps")
                    for j in range(NG):
                        for i in range(3):
                            nc.tensor.matmul(rps[:],
                                             lhsT=e8[:, j * 8:(j + 1) * 8],
                                             rhs=Ps[i][:, j * 512:(j + 1) * 512],
                                             start=(j == 0 and i == 0),
                                             stop=(j == NG - 1 and i == 2))
                    stage8 = p1sb.tile([8, 512], FR, tag="st8")
                    nc.vector.tensor_copy(stage8[:], rps[:])
                    # incremental forward-z transform: acc += Az2T[8g:8g+8].T @ stage8
                    nc.tensor.matmul(accps[:], lhsT=mats2_sb[:, g * 128:(g + 1) * 128],
                                     rhs=stage8[:], start=(g == 0), stop=(g == NG - 1))

            # ---------------- Phase 2: distributed FFT ----------------
            def cpass(fsb, fpsp, sre, sim_n, sim_, re_in, im_in, stacked=False):
                """complex pass: re_out = sre@re + sim_n@im ; im_out = sim_@re + sre@im
                (sim_n = negated imag matrix). Round-copies to f32r tiles.
                stacked=True returns one (128,512) tile [re | im] for A2A staging."""
                ps_re = fpsp.tile([N, 512], FP, tag="psre")
                ps_im = fpsp.tile([N, 512], FP, tag="psim")
                nc.tensor.matmul(ps_re[:], lhsT=sre, rhs=re_in[:], start=True, stop=False)
                nc.tensor.matmul(ps_re[:], lhsT=sim_n, rhs=im_in[:], start=False, stop=True)
                nc.tensor.matmul(ps_im[:], lhsT=sim_, rhs=re_in[:], start=True, stop=False)
                nc.tensor.matmul(ps_im[:], lhsT=sre, rhs=im_in[:], start=False, stop=True)
                if stacked:
                    o = fsb.tile([128, 512], FR, tag="fstk")
                    nc.vector.tensor_copy(o[0:N, :], ps_re[:])
                    nc.vector.tensor_copy(o[N:128, :], ps_im[:])
                    return o
                o_re = fsb.tile([N, 512], FR, tag="fre")
                o_im = fsb.tile([N, 512], FR, tag="fim")
                nc.vector.tensor_copy(o_re[:], ps_re[:])
                nc.vector.tensor_copy(o_im[:], ps_im[:])
                return o_re, o_im

            dma_engines = (nc.sync, nc.scalar, nc.gpsimd, nc.sync)

            def tstage(fsb, tps, re_in, im_in, strided_out):
                """transpose stage. Input [p | blk*64+q] (blk outer, q inner 64).
                strided_out=False: out[q | blk*64+p]  (contiguous 64-col writes)
                strided_out=True:  out[q | p*8+blk]   (stride-8 writes, 8 blocks)
                """
                o_re = fsb.tile([N, 512], FR, tag="tre")
                o_im = fsb.tile([N, 512], FR, tag="tim")
                for (src, dst) in ((re_in, o_re), (im_in, o_im)):
                    dstv = dst.rearrange("p (q b) -> p q b", q=N, b=8) if strided_out else None
                    for b in range(4):
                        pt = tps.tile([128, N], FR, tag="pt")
                        nc.tensor.transpose(pt[:], src[:, b * 128:(b + 1) * 128], ident)
                        for bb in range(2):
                            blk = 2 * b + bb
                            if strided_out:
                                nc.vector.tensor_copy(dstv[:, :, blk],
                                                      pt[bb * N:(bb + 1) * N, :])
                            else:
                                nc.vector.tensor_copy(dst[:, blk * N:(blk + 1) * N],
                                                      pt[bb * N:(bb + 1) * N, :])
                return o_re, o_im

            fsb_pool = tc.tile_pool(name="fft", bufs=2)
            fsb = fsb_pool.__enter__()
            # forward z was accumulated during phase 1; just round-copy out
            f_re = fsb.tile([N, 512], FR, tag="fre")
            f_im = fsb.tile([N, 512], FR, tag="fim")
            nc.vector.tensor_copy(f_re[:], accps[0:N, :])
            nc.vector.tensor_copy(f_im[:], accps[N:128, :])
            acc_pool.__exit__(None, None, None)
            with tc.tile_pool(name="fps", bufs=2, space="PSUM") as fps, \
                 tc.tile_pool(name="ps6p", bufs=1, space="PSUM") as ps6p, \
                 tc.tile_pool(name="tps", bufs=3, space="PSUM") as tps:
                tap("dbg_f1re", f_re); tap("dbg_f1im", f_im)
                t_re, t_im = tstage(fsb, tps, f_re, f_im, True)   # [y | kz*8+xl]
                tap("dbg_t1re", t_re); tap("dbg_t1im", t_im)
                f2_re, f2_im = cpass(fsb, fps, ArT, AiTn, AiT, t_re, t_im)  # [ky | kz*8+xl]

                tap("dbg_f2re", f2_re); tap("dbg_f2im", f2_im)
                # A2A #1: -> [ky | kzl*64 + x]   (chunk for dest d = cols [64d,64d+64))
                a_in = dram.tile([N_CORES, 2, N, 8, 8], FR, tag="a2a_in")
                a_out = dram.tile([N_CORES, 2, N, 8, 8], FR, tag="a2a_out")
                for dd in range(N_CORES):
                    nc.sync.dma_start(a_in[dd, 0], f2_re[:, dd * N:(dd + 1) * N])
                    nc.sync.dma_start(a_in[dd, 1], f2_im[:, dd * N:(dd + 1) * N])
                nc.gpsimd.collective_compute(
                    "AllToAll", mybir.AluOpType.bypass,
                    replica_groups=[list(range(N_CORES))],
                    ins=[a_in.opt()], outs=[a_out.opt()])
                # recv contiguously (cheap descriptors), then DVE-permute
                # [p | s*64+kl*8+xl] -> [p | kl*64+s*8+xl]
                g_rre = fsb.tile([N, 512], FR, tag="grre")
                g_rim = fsb.tile([N, 512], FR, tag="grim")
                for ss in range(N_CORES):
                    nc.sync.dma_start(
                        g_rre[:, ss * N:(ss + 1) * N].rearrange("p (kl xl) -> p kl xl", kl=8, xl=8),
                        a_out[ss, 0])
                    nc.sync.dma_start(
                        g_rim[:, ss * N:(ss + 1) * N].rearrange("p (kl xl) -> p kl xl", kl=8, xl=8),
                        a_out[ss, 1])
                g_re = fsb.tile([N, 512], FR, tag="fre")
                g_im = fsb.tile([N, 512], FR, tag="fim")
                nc.vector.tensor_copy(
                    g_re.rearrange("p (kl s xl) -> p s kl xl", kl=8, s=8, xl=8),
                    g_rre.rearrange("p (s kl xl) -> p s kl xl", s=8, kl=8, xl=8))
                nc.vector.tensor_copy(
                    g_im.rearrange("p (kl s xl) -> p s kl xl", kl=8, s=8, xl=8),
                    g_rim.rearrange("p (s kl xl) -> p s kl xl", s=8, kl=8, xl=8))

                tap("dbg_g2re", g_re); tap("dbg_g2im", g_im)
                t2_re, t2_im = tstage(fsb, tps, g_re, g_im, False)  # [x | kzl*64+ky]
                tap("dbg_t2re", t2_re); tap("dbg_t2im", t2_im)
                # P3 forward x, then filter fused into the PSUM->SBUF copy
                ps3_re = fps.tile([N, 512], FP, tag="psre")
                ps3_im = fps.tile([N, 512], FP, tag="psim")
                nc.tensor.matmul(ps3_re[:], lhsT=ArT, rhs=t2_re[:], start=True, stop=False)
                nc.tensor.matmul(ps3_re[:], lhsT=AiTn, rhs=t2_im[:], start=False, stop=True)
                nc.tensor.matmul(ps3_im[:], lhsT=AiT, rhs=t2_re[:], start=True, stop=False)
                nc.tensor.matmul(ps3_im[:], lhsT=ArT, rhs=t2_im[:], start=False, stop=True)
                f3_re = fsb.tile([N, 512], FR, tag="fre")
                f3_im = fsb.tile([N, 512], FR, tag="fim")
                nc.vector.tensor_tensor(f3_re[:], ps3_re[:], ham_sb[:], op=mybir.AluOpType.mult)
                nc.vector.tensor_tensor(f3_im[:], ps3_im[:], ham_sb[:], op=mybir.AluOpType.mult)

                tap("dbg_f3re", f3_re); tap("dbg_f3im", f3_im)
                g4_re, g4_im = cpass(fsb, fps, BrT, BiTn, BiT, f3_re, f3_im)  # [x | kzl*64+ky]
                t3_re, t3_im = tstage(fsb, tps, g4_re, g4_im, True)  # [ky | x*8+kzl]
                f5s = cpass(fsb, fps, BrT, BiTn, BiT, t3_re, t3_im, stacked=True)  # [y± | x*8+kzl]

                # A2A #2: -> [y | xl*64 + kz]   (chunk for dest d = cols [64d,64d+64))
                a2_in = dram.tile([N_CORES, 2, N, 8, 8], FR, tag="a2a2_in")
                a2_out = dram.tile([N_CORES, 2, N, 8, 8], FR, tag="a2a2_out")
                for dd in range(N_CORES):
                    dma_engines[dd % 4].dma_start(
                        a2_in[dd].rearrange("q p xl kl -> (q p) xl kl"),
                        f5s[:, dd * N:(dd + 1) * N].rearrange("p (xl kl) -> p xl kl", xl=8, kl=8))
                nc.gpsimd.collective_compute(
                    "AllToAll", mybir.AluOpType.bypass,
                    replica_groups=[list(range(N_CORES))],
                    ins=[a2_in.opt()], outs=[a2_out.opt()])
                # recv contiguously, then DVE-permute [p | s*64+xl*8+kl] -> [p | xl*64+s*8+kl]
                g5_raw = fsb.tile([128, 512], FR, tag="graw")
                for ss in range(N_CORES):
                    dma_engines[ss % 4].dma_start(
                        g5_raw[:, ss * N:(ss + 1) * N].rearrange("p (xl kl) -> p xl kl", xl=8, kl=8),
                        a2_out[ss].rearrange("q p xl kl -> (q p) xl kl"))
                g5_re = fsb.tile([N, 512], FR, tag="fre")
                g5_im = fsb.tile([N, 512], FR, tag="fim")
                nc.vector.tensor_copy(
                    g5_re.rearrange("p (xl s kl) -> p s xl kl", xl=8, s=8, kl=8),
                    g5_raw[0:N, :].rearrange("p (s xl kl) -> p s xl kl", s=8, xl=8, kl=8))
                nc.vector.tensor_copy(
                    g5_im.rearrange("p (xl s kl) -> p s xl kl", xl=8, s=8, kl=8),
                    g5_raw[N:128, :].rearrange("p (s xl kl) -> p s xl kl", s=8, xl=8, kl=8))

                tap("dbg_g5re", g5_re); tap("dbg_g5im", g5_im)
                t4_re, t4_im = tstage(fsb, tps, g5_re, g5_im, False)  # [kz | xl*64+y]
                # P6: inverse z, real part only
                ps6 = ps6p.tile([N, 512], FP, tag="ps6")
                nc.tensor.matmul(ps6[:], lhsT=BrT, rhs=t4_re[:], start=True, stop=False)
                nc.tensor.matmul(ps6[:], lhsT=BiTn, rhs=t4_im[:], start=False, stop=True)
                out_sb = fsb.tile([N, 512], FP, tag="osb")
                nc.vector.tensor_copy(out_sb[:], ps6[:])
                nc.sync.dma_start(out[:], out_sb[:])
            fsb_pool.__exit__(None, None, None)

    nc.compile()
    return nc


def _get_compiled():
    global _COMPILED
    if _COMPILED is None:
        _COMPILED = _build()
    return _COMPILED


def _host_inputs(X, aw, bw, real_grid_flat, hamming):
    X = np.asarray(X, np.float32)
    aw = np.asarray(aw, np.float32)
    bw = np.asarray(bw, np.float32)
    grid = np.asarray(real_grid_flat, np.float32)
    hamming = np.asarray(hamming, np.float32)

    arr = grid.reshape(N, N, N, 3)                       # [x, y, z, 3]
    arrt = np.transpose(arr, (2, 0, 1, 3))               # [z, x, y, 3]
    g2 = (arrt.astype(np.float64) ** 2).sum(-1).astype(np.float32)
    rhs5_full = np.stack(
        [arrt[..., 0], arrt[..., 1], arrt[..., 2],
         np.ones((N, N, N), np.float32), g2], 0)          # (5, z, x, y)

    xstat = np.concatenate(
        [-2.0 * X.T, (X.astype(np.float64) ** 2).sum(-1)[None, :].astype(np.float32),
         np.ones((1, A), np.float32)], 0).astype(np.float32)   # (5, 128)

    lnaw = np.log(np.maximum(aw, 1e-38)).astype(np.float32)
    bwln = np.concatenate([bw, lnaw], 1).astype(np.float32)     # (128, 12)

    F = np.fft.fft(np.eye(N), axis=0, norm='ortho')
    IF = np.fft.ifft(np.eye(N), axis=0, norm='ortho')
    perm = (np.arange(N) + N // 2) % N
    Am = F[:, perm]
    Bm = IF[perm, :]
    Ar, Ai = Am.real.astype(np.float32), Am.imag.astype(np.float32)
    Br, Bi = Bm.real.astype(np.float32), Bm.imag.astype(np.float32)
    Az2T = np.concatenate([Ar.T, Ai.T], 1)               # (64, 128)
    mats2 = np.ascontiguousarray(
        np.transpose(Az2T.reshape(8, 8, 128), (1, 0, 2))).reshape(8, 8 * 128)
    mats = np.concatenate(
        [Az2T, Ar.T, Ai.T, -Ai.T, Br.T, Bi.T, -Bi.T,
         np.eye(N, dtype=np.float32)], 1)                # (64, 576)

    Hfull = np.fft.ifftshift(hamming)                    # [kx, ky, kz]

    in_maps = []
    for c in range(N_CORES):
        rhs5c = np.ascontiguousarray(
            rhs5_full[:, :, 8 * c:8 * (c + 1), :]).reshape(5, GPC)
        Hc = np.ascontiguousarray(
            np.transpose(Hfull[:, :, 8 * c:8 * (c + 1)], (0, 2, 1))).reshape(N, 512)
        in_maps.append({"rhs5": rhs5c, "xstat": xstat, "bwln": bwln,
                        "mats": mats, "mats2": mats2, "ham": Hc})
    return in_maps


def kernel(X, aw, bw, real_grid_flat, hamming):
    global LAST_EXEC_NS, LAST_RESULTS
    in_maps = _host_inputs(X, aw, bw, real_grid_flat, hamming)
    nc = _get_compiled()

    trace = bool(os.environ.get("BASS_TRACE"))
    res = run_bass_kernel_spmd(nc, in_maps, core_ids=list(range(N_CORES)),
                               trace=trace)
    LAST_EXEC_NS = res.exec_time_ns
    global LAST_RESULTS
    LAST_RESULTS = res.results

    full = np.empty((N, N, N), np.float32)               # [z, x, y]
    for c in range(N_CORES):
        full[:, 8 * c:8 * (c + 1), :] = res.results[c]["out"].reshape(N, 8, N)
    o = np.transpose(full, (1, 2, 0))                    # [x, y, z]
    o = (o - o.mean()) / (o.std() + 1e-8)
    return o.astype(np.float32)



# revision 8
# speedup vs baseline: 2.1479x; 2.1479x over previous
"""Trainium2 Bass kernel for DensityCalculator.

density[g] = sum_a sum_k aw[a,k]*exp(bw[a,k]*|g-x_a|^2) over a 64^3 grid,
then 3D FFT -> hamming filter -> inverse FFT -> normalize.

Key identity: each (atom,k) Gaussian is separable,
  exp(bw*|g-x|^2) = exp(bw*(gx-x0)^2) * exp(bw*(gy-x1)^2) * exp(bw*(gz-x2)^2)
so the density is a sum of 768 rank-1 (outer-product) terms. The kernel
builds 1D factor tables on device (a few hundred exps instead of 25M),
z-DFTs the z-factors with one matmul per 128-chunk, forms the Khatri-Rao
product of the x/y factors, and accumulates the z-transformed density
with 6 matmuls. That replaces the whole brute-force splatting phase.

Distribution: grid x-axis sharded over 8 cores (8 x-planes each); FFT is a
distributed pencil decomposition (z,y transforms local to the x-slab,
AllToAll to kz-slabs, x transform + filter + inverse x, AllToAll back,
inverse y,z local). FFTs as matmuls with 64x64 DFT matrices
(fftshift/ifftshift folded into permutations host-side).
"""

import os
import sys
import numpy as np

for _p in ("/opt/trn_rl_repo", "/root/.axon_site", "/root/.axon_site/_ro/trn_rl_repo",
           "/root/.axon_site/_ro/pypackages"):
    if _p not in sys.path and os.path.isdir(_p):
        sys.path.append(_p)

import concourse.bass as bass
import concourse.tile as tile
from concourse import bacc, mybir
from concourse.bass_utils import run_bass_kernel_spmd

FP = mybir.dt.float32
FR = mybir.dt.float32r
BF = mybir.dt.bfloat16
Exp = mybir.ActivationFunctionType.Exp

N_CORES = 8
N = 64              # grid size per axis
A = 128             # atoms
K = 6               # gaussian terms
XL = 8              # x-planes per core
GPC = N * XL * N    # grid points per core (32768)
NG = 8              # phase-1 groups per core
GSZ = GPC // NG     # 4096 points per group

LAST_EXEC_NS = None
LAST_RESULTS = None
_COMPILED = None
DEBUG_TAPS = bool(os.environ.get("KERNEL_DEBUG_TAPS"))


def _build():
    nc = bacc.Bacc("TRN2", target_bir_lowering=False, debug=False,
                   num_devices=N_CORES)

    M = A * K  # 768 rank-1 terms, flat m = a*K + k
    mats = nc.dram_tensor("mats", [N, 128 + 7 * N], FR, kind="ExternalInput").ap()
    ham = nc.dram_tensor("ham", [N, 512], FR, kind="ExternalInput").ap()
    # z-factor build tables, [z, m] layout (z on partitions for the DFT matmul)
    zcol = nc.dram_tensor("zcol", [N, 1], FP, kind="ExternalInput").ap()
    xzb = nc.dram_tensor("xzb", [N, M], FP, kind="ExternalInput").ap()
    bwzb = nc.dram_tensor("bwzb", [N, M], FP, kind="ExternalInput").ap()
    awzb = nc.dram_tensor("awzb", [N, M], FP, kind="ExternalInput").ap()
    # xy-factor build tables, [m-chunk, 6*(8+64)] layout (m on partitions)
    xycb = nc.dram_tensor("xycb", [128, 432], FP, kind="ExternalInput").ap()
    xbxy = nc.dram_tensor("xbxy", [128, 432], FP, kind="ExternalInput").ap()
    bwbxy = nc.dram_tensor("bwbxy", [128, 432], FP, kind="ExternalInput").ap()
    out = nc.dram_tensor("out", [N, 512], FP, kind="ExternalOutput").ap()
    taps = {}
    if DEBUG_TAPS:
        for nm in ("dbg_rho", "dbg_f1re", "dbg_f1im", "dbg_t1re", "dbg_t1im",
                   "dbg_f2re", "dbg_f2im", "dbg_g2re", "dbg_g2im",
                   "dbg_t2re", "dbg_t2im", "dbg_f3re", "dbg_f3im",
                   "dbg_g5re", "dbg_g5im"):
            taps[nm] = nc.dram_tensor(nm, [N, 512], FR, kind="ExternalOutput").ap()

    def tap(nm, tile_):
        if DEBUG_TAPS:
            nc.sync.dma_start(taps[nm], tile_[:])

    with tile.TileContext(nc) as tc:
        with tc.tile_pool(name="const", bufs=1) as constp, \
             tc.tile_pool(name="dram", bufs=1, space="DRAM") as dram:
            mats_sb = constp.tile([N, 128 + 7 * N], FR)
            nc.sync.dma_start(mats_sb[:], mats[:])
            ham_sb = constp.tile([N, 512], FR)
            nc.sync.dma_start(ham_sb[:], ham[:])
            zcol_sb = constp.tile([N, 1], FP)
            nc.sync.dma_start(zcol_sb[:], zcol[:])
            xzb_sb = constp.tile([N, M], FP)
            nc.scalar.dma_start(xzb_sb[:], xzb[:])
            bwzb_sb = constp.tile([N, M], FP)
            nc.scalar.dma_start(bwzb_sb[:], bwzb[:])
            awzb_sb = constp.tile([N, M], FP)
            nc.gpsimd.dma_start(awzb_sb[:], awzb[:])
            xycb_sb = constp.tile([128, 432], FP)
            nc.gpsimd.dma_start(xycb_sb[:], xycb[:])
            xbxy_sb = constp.tile([128, 432], FP)
            nc.sync.dma_start(xbxy_sb[:], xbxy[:])
            bwbxy_sb = constp.tile([128, 432], FP)
            nc.sync.dma_start(bwbxy_sb[:], bwbxy[:])

            # tiny warmup AllToAll so ncfw channel setup overlaps phase 1
            wu_in = dram.tile([N_CORES, 8], FP, tag="wu_in")
            wu_out = dram.tile([N_CORES, 8], FP, tag="wu_out")
            wu_sb = constp.tile([1, N_CORES * 8], FP)
            nc.vector.memset(wu_sb[:], 0.0)
            nc.sync.dma_start(wu_in[:].rearrange("a b -> (a b)"), wu_sb[0, :])
            nc.gpsimd.collective_compute(
                "AllToAll", mybir.AluOpType.bypass,
                replica_groups=[list(range(N_CORES))],
                ins=[wu_in.opt()], outs=[wu_out.opt()])

            # stationary views into mats
            Az2T = mats_sb[:, 0:128]
            c0 = 128
            ArT = mats_sb[:, c0:c0 + N]
            AiT = mats_sb[:, c0 + N:c0 + 2 * N]
            AiTn = mats_sb[:, c0 + 2 * N:c0 + 3 * N]
            BrT = mats_sb[:, c0 + 3 * N:c0 + 4 * N]
            BiT = mats_sb[:, c0 + 4 * N:c0 + 5 * N]
            BiTn = mats_sb[:, c0 + 5 * N:c0 + 6 * N]
            ident = mats_sb[:, c0 + 6 * N:c0 + 7 * N]

            # ---------------- Phase 1: rank-1 separable density ----------------
            # accps[kz re|im, (xl,y)] = sum_m FzStack[m,:] (x) Rxy[m,(xl,y)]
            # FzStack = VzT.T @ Az2T (z-DFT of z-factors, fftshift folded),
            # Rxy[m,(xl,y)] = Vx[m,xl]*Vy[m,y] (Khatri-Rao, 0-stride broadcast).
            acc_pool = tc.tile_pool(name="p1acc", bufs=1, space="PSUM")
            accps = acc_pool.__enter__().tile([128, 512], FP, tag="acc")
            with tc.tile_pool(name="p1sb", bufs=1) as p1sb, \
                 tc.tile_pool(name="p1ps", bufs=2, space="PSUM") as p1ps:
                Mul = mybir.AluOpType.mult
                # z-factor table VzT[z, m] = aw*exp(bw*(zs[z]-Xz[m])^2)
                vzt = p1sb.tile([N, M], FR, tag="vzt")
                tz = p1sb.tile([N, M], FP, tag="tz")
                nc.vector.tensor_scalar(tz[:], xzb_sb[:], zcol_sb[:, 0:1], None,
                                        op0=mybir.AluOpType.subtract)
                nc.vector.tensor_tensor(tz[:], tz[:], tz[:], op=Mul)
                nc.vector.tensor_tensor(tz[:], tz[:], bwzb_sb[:], op=Mul)
                nc.scalar.activation(tz[:], tz[:], Exp)
                nc.vector.tensor_tensor(vzt[:], tz[:], awzb_sb[:], op=Mul)
                # xy-factor tables VxyT[m%128, t*72+(0:8 xl | 8:72 y)], chunk t=m//128
                vxyt = p1sb.tile([128, 432], FP, tag="vxyt")
                nc.gpsimd.tensor_tensor(vxyt[:], xycb_sb[:], xbxy_sb[:],
                                        op=mybir.AluOpType.subtract)
                nc.gpsimd.tensor_tensor(vxyt[:], vxyt[:], vxyt[:], op=Mul)
                nc.gpsimd.tensor_tensor(vxyt[:], vxyt[:], bwbxy_sb[:], op=Mul)
                nc.scalar.activation(vxyt[:], vxyt[:], Exp)
                # z-DFT each 128-chunk of VzT, then Khatri-Rao + accumulate
                Az2T_v = mats_sb[:, 0:128]
                fzs = []
                for t in range(6):
                    psf = p1ps.tile([128, 128], FP, tag="fzps")
                    nc.tensor.matmul(psf[:], lhsT=vzt[:, t * 128:(t + 1) * 128],
                                     rhs=Az2T_v, start=True, stop=True)
                    fz = p1sb.tile([128, 128], BF, tag=f"fz{t}")
                    nc.vector.tensor_copy(fz[:], psf[:])
                    fzs.append(fz)
                for t in range(6):
                    rxy = p1sb.tile([128, 512], BF, tag=f"rxy{t}")
                    rv = rxy[:].rearrange("p (xl y) -> p xl y", xl=XL, y=N)
                    vx = vxyt[:, t * 72:t * 72 + 8].unsqueeze(2).broadcast_to([128, 8, N])
                    vy = vxyt[:, t * 72 + 8:t * 72 + 72].unsqueeze(1).broadcast_to([128, 8, N])
                    eng = nc.vector if t % 2 == 0 else nc.gpsimd
                    eng.tensor_tensor(rv, vx, vy, op=Mul)
                    nc.tensor.matmul(accps[:], lhsT=fzs[t][:], rhs=rxy[:],
                                     start=(t == 0), stop=(t == 5))

            # ---------------- Phase 2: distributed FFT ----------------
            def cpass(fsb, fpsp, sre, sim_n, sim_, re_in, im_in, stacked=False):
                """complex pass: re_out = sre@re + sim_n@im ; im_out = sim_@re + sre@im
                (sim_n = negated imag matrix). Round-copies to f32r tiles.
                stacked=True returns one (128,512) tile [re | im] for A2A staging."""
                ps_re = fpsp.tile([N, 512], FP, tag="psre")
                ps_im = fpsp.tile([N, 512], FP, tag="psim")
                nc.tensor.matmul(ps_re[:], lhsT=sre, rhs=re_in[:], start=True, stop=False)
                nc.tensor.matmul(ps_re[:], lhsT=sim_n, rhs=im_in[:], start=False, stop=True)
                nc.tensor.matmul(ps_im[:], lhsT=sim_, rhs=re_in[:], start=True, stop=False)
                nc.tensor.matmul(ps_im[:], lhsT=sre, rhs=im_in[:], start=False, stop=True)
                if stacked:
                    o = fsb.tile([128, 512], FR, tag="fstk")
                    nc.vector.tensor_copy(o[0:N, :], ps_re[:])
                    nc.vector.tensor_copy(o[N:128, :], ps_im[:])
                    return o
                o_re = fsb.tile([N, 512], FR, tag="fre")
                o_im = fsb.tile([N, 512], FR, tag="fim")
                nc.vector.tensor_copy(o_re[:], ps_re[:])
                nc.vector.tensor_copy(o_im[:], ps_im[:])
                return o_re, o_im

            dma_engines = (nc.sync, nc.scalar, nc.gpsimd, nc.sync)

            def tstage(fsb, tps, re_in, im_in, strided_out):
                """transpose stage. Input [p | blk*64+q] (blk outer, q inner 64).
                strided_out=False: out[q | blk*64+p]  (contiguous 64-col writes)
                strided_out=True:  out[q | p*8+blk]   (stride-8 writes, 8 blocks)
                """
                o_re = fsb.tile([N, 512], FR, tag="tre")
                o_im = fsb.tile([N, 512], FR, tag="tim")
                for (src, dst) in ((re_in, o_re), (im_in, o_im)):
                    dstv = dst.rearrange("p (q b) -> p q b", q=N, b=8) if strided_out else None
                    for b in range(4):
                        pt = tps.tile([128, N], FR, tag="pt")
                        nc.tensor.transpose(pt[:], src[:, b * 128:(b + 1) * 128], ident)
                        for bb in range(2):
                            blk = 2 * b + bb
                            if strided_out:
                                nc.vector.tensor_copy(dstv[:, :, blk],
                                                      pt[bb * N:(bb + 1) * N, :])
                            else:
                                nc.vector.tensor_copy(dst[:, blk * N:(blk + 1) * N],
                                                      pt[bb * N:(bb + 1) * N, :])
                return o_re, o_im

            fsb_pool = tc.tile_pool(name="fft", bufs=2)
            fsb = fsb_pool.__enter__()
            # forward z was accumulated during phase 1; just round-copy out
            f_re = fsb.tile([N, 512], FR, tag="fre")
            f_im = fsb.tile([N, 512], FR, tag="fim")
            nc.vector.tensor_copy(f_re[:], accps[0:N, :])
            nc.vector.tensor_copy(f_im[:], accps[N:128, :])
            acc_pool.__exit__(None, None, None)
            with tc.tile_pool(name="fps", bufs=2, space="PSUM") as fps, \
                 tc.tile_pool(name="ps6p", bufs=1, space="PSUM") as ps6p, \
                 tc.tile_pool(name="tps", bufs=3, space="PSUM") as tps:
                tap("dbg_f1re", f_re); tap("dbg_f1im", f_im)
                t_re, t_im = tstage(fsb, tps, f_re, f_im, True)   # [y | kz*8+xl]
                tap("dbg_t1re", t_re); tap("dbg_t1im", t_im)
                f2_re, f2_im = cpass(fsb, fps, ArT, AiTn, AiT, t_re, t_im)  # [ky | kz*8+xl]

                tap("dbg_f2re", f2_re); tap("dbg_f2im", f2_im)
                # A2A #1: -> [ky | kzl*64 + x]   (chunk for dest d = cols [64d,64d+64))
                a_in = dram.tile([N_CORES, 2, N, 8, 8], FR, tag="a2a_in")
                a_out = dram.tile([N_CORES, 2, N, 8, 8], FR, tag="a2a_out")
                for dd in range(N_CORES):
                    nc.sync.dma_start(a_in[dd, 0], f2_re[:, dd * N:(dd + 1) * N])
                    nc.sync.dma_start(a_in[dd, 1], f2_im[:, dd * N:(dd + 1) * N])
                nc.gpsimd.collective_compute(
                    "AllToAll", mybir.AluOpType.bypass,
                    replica_groups=[list(range(N_CORES))],
                    ins=[a_in.opt()], outs=[a_out.opt()])
                # recv contiguously (cheap descriptors), then DVE-permute
                # [p | s*64+kl*8+xl] -> [p | kl*64+s*8+xl]
                g_rre = fsb.tile([N, 512], FR, tag="grre")
                g_rim = fsb.tile([N, 512], FR, tag="grim")
                for ss in range(N_CORES):
                    nc.sync.dma_start(
                        g_rre[:, ss * N:(ss + 1) * N].rearrange("p (kl xl) -> p kl xl", kl=8, xl=8),
                        a_out[ss, 0])
                    nc.sync.dma_start(
                        g_rim[:, ss * N:(ss + 1) * N].rearrange("p (kl xl) -> p kl xl", kl=8, xl=8),
                        a_out[ss, 1])
                g_re = fsb.tile([N, 512], FR, tag="fre")
                g_im = fsb.tile([N, 512], FR, tag="fim")
                nc.vector.tensor_copy(
                    g_re.rearrange("p (kl s xl) -> p s kl xl", kl=8, s=8, xl=8),
                    g_rre.rearrange("p (s kl xl) -> p s kl xl", s=8, kl=8, xl=8))
                nc.vector.tensor_copy(
                    g_im.rearrange("p (kl s xl) -> p s kl xl", kl=8, s=8, xl=8),
                    g_rim.rearrange("p (s kl xl) -> p s kl xl", s=8, kl=8, xl=8))

                tap("dbg_g2re", g_re); tap("dbg_g2im", g_im)
                t2_re, t2_im = tstage(fsb, tps, g_re, g_im, False)  # [x | kzl*64+ky]
                tap("dbg_t2re", t2_re); tap("dbg_t2im", t2_im)
                # P3 forward x, then filter fused into the PSUM->SBUF copy
                ps3_re = fps.tile([N, 512], FP, tag="psre")
                ps3_im = fps.tile([N, 512], FP, tag="psim")
                nc.tensor.matmul(ps3_re[:], lhsT=ArT, rhs=t2_re[:], start=True, stop=False)
                nc.tensor.matmul(ps3_re[:], lhsT=AiTn, rhs=t2_im[:], start=False, stop=True)
                nc.tensor.matmul(ps3_im[:], lhsT=AiT, rhs=t2_re[:], start=True, stop=False)
                nc.tensor.matmul(ps3_im[:], lhsT=ArT, rhs=t2_im[:], start=False, stop=True)
                f3_re = fsb.tile([N, 512], FR, tag="fre")
                f3_im = fsb.tile([N, 512], FR, tag="fim")
                nc.vector.tensor_tensor(f3_re[:], ps3_re[:], ham_sb[:], op=mybir.AluOpType.mult)
                nc.vector.tensor_tensor(f3_im[:], ps3_im[:], ham_sb[:], op=mybir.AluOpType.mult)

                tap("dbg_f3re", f3_re); tap("dbg_f3im", f3_im)
                g4_re, g4_im = cpass(fsb, fps, BrT, BiTn, BiT, f3_re, f3_im)  # [x | kzl*64+ky]
                t3_re, t3_im = tstage(fsb, tps, g4_re, g4_im, True)  # [ky | x*8+kzl]
                f5s = cpass(fsb, fps, BrT, BiTn, BiT, t3_re, t3_im, stacked=True)  # [y± | x*8+kzl]

                # A2A #2: -> [y | xl*64 + kz]   (chunk for dest d = cols [64d,64d+64))
                a2_in = dram.tile([N_CORES, 2, N, 8, 8], FR, tag="a2a2_in")
                a2_out = dram.tile([N_CORES, 2, N, 8, 8], FR, tag="a2a2_out")
                for dd in range(N_CORES):
                    dma_engines[dd % 4].dma_start(
                        a2_in[dd].rearrange("q p xl kl -> (q p) xl kl"),
                        f5s[:, dd * N:(dd + 1) * N].rearrange("p (xl kl) -> p xl kl", xl=8, kl=8))
                nc.gpsimd.collective_compute(
                    "AllToAll", mybir.AluOpType.bypass,
                    replica_groups=[list(range(N_CORES))],
                    ins=[a2_in.opt()], outs=[a2_out.opt()])
                # recv contiguously, then DVE-permute [p | s*64+xl*8+kl] -> [p | xl*64+s*8+kl]
                g5_raw = fsb.tile([128, 512], FR, tag="graw")
                for ss in range(N_CORES):
                    dma_engines[ss % 4].dma_start(
                        g5_raw[:, ss * N:(ss + 1) * N].rearrange("p (xl kl) -> p xl kl", xl=8, kl=8),
                        a2_out[ss].rearrange("q p xl kl -> (q p) xl kl"))
                g5_re = fsb.tile([N, 512], FR, tag="fre")
                g5_im = fsb.tile([N, 512], FR, tag="fim")
                nc.vector.tensor_copy(
                    g5_re.rearrange("p (xl s kl) -> p s xl kl", xl=8, s=8, kl=8),
                    g5_raw[0:N, :].rearrange("p (s xl kl) -> p s xl kl", s=8, xl=8, kl=8))
                nc.vector.tensor_copy(
                    g5_im.rearrange("p (xl s kl) -> p s xl kl", xl=8, s=8, kl=8),
                    g5_raw[N:128, :].rearrange("p (s xl kl) -> p s xl kl", s=8, xl=8, kl=8))

                tap("dbg_g5re", g5_re); tap("dbg_g5im", g5_im)
                t4_re, t4_im = tstage(fsb, tps, g5_re, g5_im, False)  # [kz | xl*64+y]
                # P6: inverse z, real part only
                ps6 = ps6p.tile([N, 512], FP, tag="ps6")
                nc.tensor.matmul(ps6[:], lhsT=BrT, rhs=t4_re[:], start=True, stop=False)
                nc.tensor.matmul(ps6[:], lhsT=BiTn, rhs=t4_im[:], start=False, stop=True)
                out_sb = fsb.tile([N, 512], FP, tag="osb")
                nc.vector.tensor_copy(out_sb[:], ps6[:])
                nc.sync.dma_start(out[:], out_sb[:])
            fsb_pool.__exit__(None, None, None)

    nc.compile()
    return nc


def _get_compiled():
    global _COMPILED
    if _COMPILED is None:
        _COMPILED = _build()
    return _COMPILED


def _host_inputs(X, aw, bw, real_grid_flat, hamming):
    X = np.asarray(X, np.float32)
    aw = np.asarray(aw, np.float32)
    bw = np.asarray(bw, np.float32)
    grid = np.asarray(real_grid_flat, np.float32)
    hamming = np.asarray(hamming, np.float32)

    arr = grid.reshape(N, N, N, 3)                       # [x, y, z, 3]
    xs = np.ascontiguousarray(arr[:, 0, 0, 0])           # axis coordinate vectors
    ys = np.ascontiguousarray(arr[0, :, 0, 1])
    zs = np.ascontiguousarray(arr[0, 0, :, 2])

    M = A * K                                            # flat m = a*K + k
    Xf = np.repeat(X, K, axis=0)                         # (768, 3)
    bwf = bw.reshape(M)
    awf = aw.reshape(M)

    # z-factor build tables, [z, m]
    xzb = np.ascontiguousarray(np.broadcast_to(Xf[:, 2], (N, M)), np.float32)
    bwzb = np.ascontiguousarray(np.broadcast_to(bwf, (N, M)), np.float32)
    awzb = np.ascontiguousarray(np.broadcast_to(awf, (N, M)), np.float32)
    zcol = zs.reshape(N, 1).astype(np.float32)

    # xy-factor build tables, [m%128, t*72 + (0:8 xl | 8:72 y)], t = m//128
    xbxy = np.empty((128, 432), np.float32)
    bwbxy = np.empty((128, 432), np.float32)
    xycb_cores = []
    for t in range(6):
        ms = np.arange(t * 128, (t + 1) * 128)
        xbxy[:, t * 72:t * 72 + 8] = Xf[ms, 0:1]
        xbxy[:, t * 72 + 8:t * 72 + 72] = Xf[ms, 1:2]
        bwbxy[:, t * 72:t * 72 + 72] = bwf[ms, None]

    F = np.fft.fft(np.eye(N), axis=0, norm='ortho')
    IF = np.fft.ifft(np.eye(N), axis=0, norm='ortho')
    perm = (np.arange(N) + N // 2) % N
    Am = F[:, perm]
    Bm = IF[perm, :]
    Ar, Ai = Am.real.astype(np.float32), Am.imag.astype(np.float32)
    Br, Bi = Bm.real.astype(np.float32), Bm.imag.astype(np.float32)
    Az2T = np.concatenate([Ar.T, Ai.T], 1)               # (64, 128)
    mats = np.concatenate(
        [Az2T, Ar.T, Ai.T, -Ai.T, Br.T, Bi.T, -Bi.T,
         np.eye(N, dtype=np.float32)], 1)                # (64, 576)

    Hfull = np.fft.ifftshift(hamming)                    # [kx, ky, kz]

    in_maps = []
    for c in range(N_CORES):
        xyc = np.empty((128, 432), np.float32)
        for t in range(6):
            xyc[:, t * 72:t * 72 + 8] = xs[8 * c:8 * (c + 1)][None, :]
            xyc[:, t * 72 + 8:t * 72 + 72] = ys[None, :]
        Hc = np.ascontiguousarray(
            np.transpose(Hfull[:, :, 8 * c:8 * (c + 1)], (0, 2, 1))).reshape(N, 512)
        in_maps.append({"mats": mats, "ham": Hc, "zcol": zcol,
                        "xzb": xzb, "bwzb": bwzb, "awzb": awzb,
                        "xycb": xyc, "xbxy": xbxy, "bwbxy": bwbxy})
    return in_maps


def kernel(X, aw, bw, real_grid_flat, hamming):
    global LAST_EXEC_NS, LAST_RESULTS
    in_maps = _host_inputs(X, aw, bw, real_grid_flat, hamming)
    nc = _get_compiled()

    trace = bool(os.environ.get("BASS_TRACE"))
    res = run_bass_kernel_spmd(nc, in_maps, core_ids=list(range(N_CORES)),
                               trace=trace)
    LAST_EXEC_NS = res.exec_time_ns
    global LAST_RESULTS
    LAST_RESULTS = res.results

    full = np.empty((N, N, N), np.float32)               # [z, x, y]
    for c in range(N_CORES):
        full[:, 8 * c:8 * (c + 1), :] = res.results[c]["out"].reshape(N, 8, N)
    o = np.transpose(full, (1, 2, 0))                    # [x, y, z]
    o = (o - o.mean()) / (o.std() + 1e-8)
    return o.astype(np.float32)



# revision 23
# speedup vs baseline: 2.6225x; 1.2210x over previous
"""Trainium2 Bass kernel for DensityCalculator.

density[g] = sum_a sum_k aw[a,k]*exp(bw[a,k]*|g-x_a|^2) over a 64^3 grid,
then 3D FFT -> hamming filter -> inverse FFT -> normalize.

Key identities exploited:
 - Each (atom,k) Gaussian is separable: a sum of 768 rank-1 terms.
 - The DFT of a rank-1 term is the outer product of the 1D DFTs of its
   factors, so the *filtered spectrum* is built directly in Fourier
   space: no forward FFT and no first AllToAll at all.

Per core c (owns kz-slab [8c,8c+8)):
 1. build 1D factor tables Vx,Vy,Vz on device (a few hundred exps),
 2. DFT them with one matmul per 128-chunk (Fx stacked re|im for the
    stage-1 lhsT, Fy full, Fz only the local 8 kz columns),
 3. R = Fy (x) Fz_local (complex Khatri-Rao, 0-stride broadcast APs),
 4. F_hat[kx re|im, (kzl,ky)] = sum_m Fx (x) R via 12 accumulating
    matmuls (complex folded into two stacked lhsT weight sets),
 5. multiply by hamming -> f3, inverse-x and inverse-y transforms
    (stacked complex matmuls + PE transposes), pre-transpose for the
    A2A so the receive side needs no transpose,
 6. single AllToAll to x-slabs, inverse-z (real part), write out.

A tiny warmup AllToAll is issued first so the ~40us collective
channel-setup barrier overlaps all of the compute.
"""

import os
import sys
import numpy as np

for _p in ("/opt/trn_rl_repo", "/root/.axon_site", "/root/.axon_site/_ro/trn_rl_repo",
           "/root/.axon_site/_ro/pypackages"):
    if _p not in sys.path and os.path.isdir(_p):
        sys.path.append(_p)

import concourse.bass as bass
import concourse.tile as tile
from concourse import bacc, mybir
from concourse.bass_utils import run_bass_kernel_spmd

FP = mybir.dt.float32
FR = mybir.dt.float32r
BF = mybir.dt.bfloat16
Exp = mybir.ActivationFunctionType.Exp

N_CORES = 8
N = 64              # grid size per axis
A = 128             # atoms
K = 6               # gaussian terms
M = A * K           # 768 rank-1 terms, flat m = a*K + k
XL = 8              # x-planes per core in the output sharding

LAST_EXEC_NS = None
LAST_RESULTS = None
_COMPILED = None


def _build():
    nc = bacc.Bacc("TRN2", target_bir_lowering=False, debug=False,
                   num_devices=N_CORES)

    mats = nc.dram_tensor("mats", [N, 832], FR, kind="ExternalInput").ap()
    idt = nc.dram_tensor("idt", [128, 128], FR, kind="ExternalInput").ap()
    ham = nc.dram_tensor("ham", [N, 512], FR, kind="ExternalInput").ap()
    azfwd = nc.dram_tensor("azfwd", [N, 144], BF, kind="ExternalInput").ap()
    # z-factor build tables, [z, m] layout (z on partitions for the DFT)
    zcol = nc.dram_tensor("zcol", [N, 1], FP, kind="ExternalInput").ap()
    xzb = nc.dram_tensor("xzb", [N, M], FP, kind="ExternalInput").ap()
    bwzb = nc.dram_tensor("bwzb", [N, M], FP, kind="ExternalInput").ap()
    awzb = nc.dram_tensor("awzb", [N, M], BF, kind="ExternalInput").ap()
    # xy-factor build tables, [(x;y) coord, m] layout
    ccol = nc.dram_tensor("ccol", [128, 1], FP, kind="ExternalInput").ap()
    xxyb = nc.dram_tensor("xxyb", [128, M], FP, kind="ExternalInput").ap()
    bwxyb = nc.dram_tensor("bwxyb", [128, M], FP, kind="ExternalInput").ap()
    out = nc.dram_tensor("out", [N, 512], FP, kind="ExternalOutput").ap()

    with tile.TileContext(nc) as tc:
        with tc.tile_pool(name="const", bufs=1) as constp, \
             tc.tile_pool(name="dram", bufs=1, space="DRAM") as dram:
            # warmup AllToAll issued first: overlaps the collective
            # channel-setup barrier with all of the compute below.
            wu_in = dram.tile([N_CORES, 8], FP, tag="wu_in")
            wu_out = dram.tile([N_CORES, 8], FP, tag="wu_out")
            wu_sb = constp.tile([1, N_CORES * 8], FP)
            nc.vector.memset(wu_sb[:], 0.0)
            nc.sync.dma_start(wu_in[:].rearrange("a b -> (a b)"), wu_sb[0, :])
            nc.gpsimd.collective_compute(
                "AllToAll", mybir.AluOpType.bypass,
                replica_groups=[list(range(N_CORES))],
                ins=[wu_in.opt()], outs=[wu_out.opt()])

            mats_sb = constp.tile([N, 832], FR)
            nc.sync.dma_start(mats_sb[:], mats[:])
            idt_sb = constp.tile([128, 128], FR)
            nc.sync.dma_start(idt_sb[:], idt[:])
            ham_sb = constp.tile([N, 512], FR)
            nc.sync.dma_start(ham_sb[:], ham[:])
            azfwd_sb = constp.tile([N, 144], BF)
            nc.sync.dma_start(azfwd_sb[:], azfwd[:])
            zcol_sb = constp.tile([N, 1], FP)
            nc.sync.dma_start(zcol_sb[:], zcol[:])
            ccol_sb = constp.tile([128, 1], FP)
            nc.sync.dma_start(ccol_sb[:], ccol[:])
            xzb_sb = constp.tile([N, M], FP)
            nc.scalar.dma_start(xzb_sb[:], xzb[:])
            bwzb_sb = constp.tile([N, M], FP)
            nc.scalar.dma_start(bwzb_sb[:], bwzb[:])
            awzb_sb = constp.tile([N, M], BF)
            nc.scalar.dma_start(awzb_sb[:], awzb[:])
            xxyb_sb = constp.tile([128, M], FP)
            nc.sync.dma_start(xxyb_sb[:], xxyb[:])
            bwxyb_sb = constp.tile([128, M], FP)
            nc.sync.dma_start(bwxyb_sb[:], bwxyb[:])

            # stationary views into mats
            Az2T = azfwd_sb[:, 0:128]            # [Ar.T | Ai.T] forward DFT, bf16
            Azloc = azfwd_sb[:, 128:144]         # local kz columns of Az2T
            c0 = 128
            BrT = mats_sb[:, c0 + 3 * N:c0 + 4 * N]
            BiTn = mats_sb[:, c0 + 5 * N:c0 + 6 * N]
            S1 = mats_sb[:, 576:704]             # [Br.T | Bi.T]
            S2 = mats_sb[:, 704:832]             # [-Bi.T | Br.T]

            Mul = mybir.AluOpType.mult
            Sub = mybir.AluOpType.subtract
            Add = mybir.AluOpType.add

            with tc.tile_pool(name="p1sb", bufs=1) as p1sb, \
                 tc.tile_pool(name="fsb", bufs=1) as fsb, \
                 tc.tile_pool(name="p1ps", bufs=2, space="PSUM") as p1ps, \
                 tc.tile_pool(name="accp", bufs=1, space="PSUM") as accp, \
                 tc.tile_pool(name="fps", bufs=2, space="PSUM") as fps, \
                 tc.tile_pool(name="tps", bufs=2, space="PSUM") as tps:
                # ---- 1. factor tables (f32 build chain, bf16 exp output) ----
                # vxyT[(x;y), m] = exp(bw[m]*(c-X[m,axis])^2), axis by row half
                vxyf = p1sb.tile([128, M], FP, tag="vxyf")
                nc.vector.tensor_scalar(vxyf[:], xxyb_sb[:], ccol_sb[:, 0:1], None,
                                        op0=Sub)
                nc.vector.tensor_tensor(vxyf[:], vxyf[:], vxyf[:], op=Mul)
                nc.vector.tensor_tensor(vxyf[:], vxyf[:], bwxyb_sb[:], op=Mul)
                vxyt = p1sb.tile([128, M], BF, tag="vxyt")
                nc.scalar.activation(vxyt[:], vxyf[:], Exp)
                # y-half copied to base-partition 0 (matmul operands must align)
                vyt = p1sb.tile([N, M], BF, tag="vyt")
                nc.vector.tensor_copy(vyt[:], vxyt[N:128, :])
                # vzT[z, m] = aw[m]*exp(bw[m]*(zs-X[m,2])^2)
                vzf = p1sb.tile([N, M], FP, tag="vzf")
                nc.vector.tensor_scalar(vzf[:], xzb_sb[:], zcol_sb[:, 0:1], None,
                                        op0=Sub)
                nc.vector.tensor_tensor(vzf[:], vzf[:], vzf[:], op=Mul)
                nc.vector.tensor_tensor(vzf[:], vzf[:], bwzb_sb[:], op=Mul)
                vze = p1sb.tile([N, M], BF, tag="vze")
                nc.scalar.activation(vze[:], vzf[:], Exp)
                vzt = p1sb.tile([N, M], BF, tag="vzt")
                nc.vector.tensor_tensor(vzt[:], vze[:], awzb_sb[:], op=Mul)

                # ---- 2. factor DFTs per 128-chunk ----
                stkA, stkB, fys, fzs = [], [], [], []
                for t in range(6):
                    ch = slice(t * 128, (t + 1) * 128)
                    psd = p1ps.tile([128, 272], FP, tag="dft")
                    nc.tensor.matmul(psd[:, 0:128], lhsT=vxyt[0:N, ch], rhs=Az2T,
                                     start=True, stop=True)
                    nc.tensor.matmul(psd[:, 128:256], lhsT=vyt[:, ch], rhs=Az2T,
                                     start=True, stop=True)
                    nc.tensor.matmul(psd[:, 256:272], lhsT=vzt[:, ch], rhs=Azloc,
                                     start=True, stop=True)
                    sa = p1sb.tile([128, 128], BF, tag=f"sa{t}")
                    nc.scalar.copy(sa[:], psd[:, 0:128])
                    sb = p1sb.tile([128, 128], BF, tag=f"sb{t}")
                    nc.vector.tensor_scalar(sb[:, 0:N], psd[:, N:128], -1.0, None,
                                            op0=Mul)
                    nc.vector.tensor_copy(sb[:, N:128], psd[:, 0:N])
                    stkA.append(sa)
                    stkB.append(sb)
                    fy = p1sb.tile([128, 128], BF, tag=f"fy{t}")
                    nc.scalar.copy(fy[:], psd[:, 128:256])
                    fys.append(fy)
                    fz = p1sb.tile([128, 16], BF, tag=f"fz{t}")
                    nc.vector.tensor_copy(fz[:], psd[:, 256:272])
                    fzs.append(fz)

                # ---- 3+4. R = Fy (x) Fz_local, F_hat accumulation ----
                accps = accp.tile([128, 512], FP, tag="acc")
                for t in range(6):
                    fy, fz = fys[t], fzs[t]
                    fyR = fy[:, 0:N].unsqueeze(1).broadcast_to([128, 8, N])
                    fyI = fy[:, N:128].unsqueeze(1).broadcast_to([128, 8, N])
                    fzR = fz[:, 0:8].unsqueeze(2).broadcast_to([128, 8, N])
                    fzI = fz[:, 8:16].unsqueeze(2).broadcast_to([128, 8, N])
                    m1 = p1sb.tile([128, 512], BF, tag="m1")
                    m2 = p1sb.tile([128, 512], BF, tag="m2")
                    rre = p1sb.tile([128, 512], BF, tag=f"rre{t}")
                    rim = p1sb.tile([128, 512], BF, tag=f"rim{t}")
                    m1v = m1[:].rearrange("p (a b) -> p a b", a=8, b=N)
                    m2v = m2[:].rearrange("p (a b) -> p a b", a=8, b=N)
                    rrev = rre[:].rearrange("p (a b) -> p a b", a=8, b=N)
                    rimv = rim[:].rearrange("p (a b) -> p a b", a=8, b=N)
                    nc.vector.tensor_tensor(m1v, fyR, fzR, op=Mul)
                    nc.vector.tensor_tensor(m2v, fyI, fzI, op=Mul)
                    nc.vector.tensor_tensor(rrev, m1v, m2v, op=Sub)
                    nc.vector.tensor_tensor(m1v, fyR, fzI, op=Mul)
                    nc.vector.tensor_tensor(m2v, fyI, fzR, op=Mul)
                    nc.vector.tensor_tensor(rimv, m1v, m2v, op=Add)
                    nc.tensor.matmul(accps[:], lhsT=stkA[t][:], rhs=rre[:],
                                     start=(t == 0), stop=False)
                    nc.tensor.matmul(accps[:], lhsT=stkB[t][:], rhs=rim[:],
                                     start=False, stop=(t == 5))

                # ---- 5. hamming, inverse x, transpose, inverse y ----
                f3_re = fsb.tile([N, 512], FR, tag="f3re")
                f3_im = fsb.tile([N, 512], FR, tag="f3im")
                nc.vector.tensor_tensor(f3_re[:], accps[0:N, :], ham_sb[:], op=Mul)
                nc.vector.tensor_tensor(f3_im[:], accps[N:128, :], ham_sb[:], op=Mul)

                # g4 = inverse-x, stacked [x-re | x-im] in one psum
                ps4 = fps.tile([128, 512], FP, tag="cps")
                nc.tensor.matmul(ps4[:], lhsT=S1, rhs=f3_re[:], start=True, stop=False)
                nc.tensor.matmul(ps4[:], lhsT=S2, rhs=f3_im[:], start=False, stop=True)
                g4 = fsb.tile([128, 512], FR, tag="g4")
                nc.scalar.copy(g4[0:N, :], ps4[0:N, :])
                nc.vector.tensor_copy(g4[N:128, :], ps4[N:128, :])

                # t3: [x-re|x-im, (kzl,ky)] -> [ky, (x,kzl)] re and im
                # (x outer so each A2A dest's 64 columns are contiguous)
                t3_re = fsb.tile([N, 512], FR, tag="t3re")
                t3_im = fsb.tile([N, 512], FR, tag="t3im")
                t3rv = t3_re[:].rearrange("p (x kzl) -> p x kzl", x=N, kzl=8)
                t3iv = t3_im[:].rearrange("p (x kzl) -> p x kzl", x=N, kzl=8)
                for b in range(4):
                    pt = tps.tile([128, 128], FR, tag="pt")
                    nc.tensor.transpose(pt[:], g4[:, b * 128:(b + 1) * 128], idt_sb[:])
                    for bb in range(2):
                        kzl = 2 * b + bb
                        nc.vector.tensor_copy(
                            t3rv[:, :, kzl], pt[bb * N:(bb + 1) * N, 0:N])
                        nc.vector.tensor_copy(
                            t3iv[:, :, kzl], pt[bb * N:(bb + 1) * N, N:128])

                # f5 = inverse-y, stacked [y-re | y-im]
                ps5 = fps.tile([128, 512], FP, tag="cps")
                nc.tensor.matmul(ps5[:], lhsT=S1, rhs=t3_re[:], start=True, stop=False)
                nc.tensor.matmul(ps5[:], lhsT=S2, rhs=t3_im[:], start=False, stop=True)
                f5 = fsb.tile([128, 512], FR, tag="f5")
                nc.scalar.copy(f5[0:N, :], ps5[0:N, :])
                nc.vector.tensor_copy(f5[N:128, :], ps5[N:128, :])

                # ---- 6. pre-transposed AllToAll to x-slabs ----
                # f5 cols are (x,kzl); for dest d transpose its contiguous
                # 64-col block -> [(xl,kzl), y-re|y-im]: receive needs no
                # transpose, only a strided DMA.
                a2_in = dram.tile([N_CORES, N, 128], FR, tag="a2a2_in")
                a2_out = dram.tile([N_CORES, N, 128], FR, tag="a2a2_out")
                dma_engines = (nc.sync, nc.scalar, nc.sync, nc.scalar)
                for dd in range(N_CORES):
                    ptd = tps.tile([128, 128], FR, tag="pt")
                    nc.tensor.transpose(ptd[0:N, :], f5[:, dd * N:(dd + 1) * N],
                                        idt_sb[:])
                    std = fsb.tile([N, 128], FR, tag="std")
                    nc.vector.tensor_copy(std[:], ptd[0:N, :])
                    dma_engines[dd % 4].dma_start(a2_in[dd], std[:])
                nc.gpsimd.collective_compute(
                    "AllToAll", mybir.AluOpType.bypass,
                    replica_groups=[list(range(N_CORES))],
                    ins=[a2_in.opt()], outs=[a2_out.opt()])
                # receive: src ss rows (xl,kzl) cols (y-re|y-im)
                # -> g6_re/g6_im [kz, xl*64+y]
                g6_re = fsb.tile([N, 512], FR, tag="g6re")
                g6_im = fsb.tile([N, 512], FR, tag="g6im")
                for ss in range(N_CORES):
                    src = a2_out[ss].rearrange("(xl kzl) c -> kzl xl c", xl=8, kzl=8)
                    dst_re = g6_re[ss * 8:(ss + 1) * 8, :].rearrange(
                        "p (xl y) -> p xl y", xl=8, y=N)
                    dst_im = g6_im[ss * 8:(ss + 1) * 8, :].rearrange(
                        "p (xl y) -> p xl y", xl=8, y=N)
                    dma_engines[ss % 4].dma_start(dst_re, src[:, :, 0:N])
                    dma_engines[(ss + 1) % 4].dma_start(dst_im, src[:, :, N:128])

                # inverse-z, real part only
                ps6 = fps.tile([128, 512], FP, tag="cps")
                nc.tensor.matmul(ps6[0:N, :], lhsT=BrT, rhs=g6_re[:], start=True, stop=False)
                nc.tensor.matmul(ps6[0:N, :], lhsT=BiTn, rhs=g6_im[:], start=False, stop=True)
                out_sb = fsb.tile([N, 512], FP, tag="osb")
                nc.vector.tensor_copy(out_sb[:], ps6[0:N, :])
                nc.sync.dma_start(out[:], out_sb[:])

    nc.compile()
    return nc


def _get_compiled():
    global _COMPILED
    if _COMPILED is None:
        _COMPILED = _build()
    return _COMPILED


def _host_inputs(X, aw, bw, real_grid_flat, hamming):
    X = np.asarray(X, np.float32)
    aw = np.asarray(aw, np.float32)
    bw = np.asarray(bw, np.float32)
    grid = np.asarray(real_grid_flat, np.float32)
    hamming = np.asarray(hamming, np.float32)

    arr = grid.reshape(N, N, N, 3)                       # [x, y, z, 3]
    xs = np.ascontiguousarray(arr[:, 0, 0, 0])           # axis coordinate vectors
    ys = np.ascontiguousarray(arr[0, :, 0, 1])
    zs = np.ascontiguousarray(arr[0, 0, :, 2])

    Xf = np.repeat(X, K, axis=0)                         # (768, 3)
    bwf = bw.reshape(M)
    awf = aw.reshape(M)

    import ml_dtypes
    bf16 = ml_dtypes.bfloat16

    # z-factor build tables, [z, m]
    xzb = np.ascontiguousarray(np.broadcast_to(Xf[:, 2], (N, M)), np.float32)
    bwzb = np.ascontiguousarray(np.broadcast_to(bwf, (N, M)), np.float32)
    awzb = np.ascontiguousarray(np.broadcast_to(awf, (N, M))).astype(bf16)
    zcol = zs.reshape(N, 1).astype(np.float32)

    # xy-factor build tables, [(x;y), m]
    ccol = np.concatenate([xs, ys]).reshape(128, 1).astype(np.float32)
    xxyb = np.empty((128, M), np.float32)
    xxyb[0:N, :] = Xf[:, 0][None, :]
    xxyb[N:128, :] = Xf[:, 1][None, :]
    bwxyb = np.ascontiguousarray(np.broadcast_to(bwf, (128, M)), np.float32)

    F = np.fft.fft(np.eye(N), axis=0, norm='ortho')
    IF = np.fft.ifft(np.eye(N), axis=0, norm='ortho')
    perm = (np.arange(N) + N // 2) % N
    Am = F[:, perm]
    Bm = IF[perm, :]
    Ar, Ai = Am.real.astype(np.float32), Am.imag.astype(np.float32)
    Br, Bi = Bm.real.astype(np.float32), Bm.imag.astype(np.float32)
    Az2T = np.concatenate([Ar.T, Ai.T], 1)               # (64, 128)
    mats = np.concatenate(
        [Az2T, Ar.T, Ai.T, -Ai.T, Br.T, Bi.T, -Bi.T,
         np.eye(N, dtype=np.float32),
         Br.T, Bi.T, -Bi.T, Br.T], 1)                    # (64, 832)

    Hfull = np.fft.ifftshift(hamming)                    # [kx, ky, kz]
    idt = np.eye(128, dtype=np.float32)

    in_maps = []
    for c in range(N_CORES):
        Hc = np.ascontiguousarray(
            np.transpose(Hfull[:, :, 8 * c:8 * (c + 1)], (0, 2, 1))).reshape(N, 512)
        azfwd = np.concatenate(
            [Az2T, Az2T[:, 8 * c:8 * (c + 1)], Az2T[:, N + 8 * c:N + 8 * (c + 1)]],
            axis=1).astype(bf16)                         # (64, 144)
        in_maps.append({"mats": mats, "idt": idt, "ham": Hc, "azfwd": azfwd,
                        "zcol": zcol, "xzb": xzb, "bwzb": bwzb, "awzb": awzb,
                        "ccol": ccol, "xxyb": xxyb, "bwxyb": bwxyb})
    return in_maps


def kernel(X, aw, bw, real_grid_flat, hamming):
    global LAST_EXEC_NS, LAST_RESULTS
    in_maps = _host_inputs(X, aw, bw, real_grid_flat, hamming)
    nc = _get_compiled()

    trace = bool(os.environ.get("BASS_TRACE"))
    res = run_bass_kernel_spmd(nc, in_maps, core_ids=list(range(N_CORES)),
                               trace=trace)
    LAST_EXEC_NS = res.exec_time_ns
    LAST_RESULTS = res.results

    full = np.empty((N, N, N), np.float32)               # [z, x, y]
    for c in range(N_CORES):
        full[:, 8 * c:8 * (c + 1), :] = res.results[c]["out"].reshape(N, 8, N)
    o = np.transpose(full, (1, 2, 0))                    # [x, y, z]
    o = (o - o.mean()) / (o.std() + 1e-8)
    return o.astype(np.float32)
